# revision 9
# baseline (speedup 1.0000x reference)
"""BatchHardContrastiveLoss Trainium2 kernel (8-core SPMD), v4.

Math: with rows/columns class-sorted (labels recovered on the host from the
mask structure), the PE emits ONE biased matrix per anchor tile:

    v[i,j] = s*(d2(i,j) - sq_i) - A + B*same[i,j]

The B*same band (classes are contiguous after sorting, so same(i,j) for a
128-row block lives in a fixed 256-wide diagonal window) is added by a tiny
rank-<=64 one-hot fp8 matmul accumulated onto the main DoubleRow matmul's
PSUM segments.  B > 5*s*sq_max guarantees strict separation, so

    row max of v  = B - A + s*(hardest_pos_d2 - sq_i)   (pos mining, diag incl.)
    row min of v  =   - A + s*(hardest_neg_d2 - sq_i)   (neg mining)

i.e. both reductions are PLAIN unmasked max/min — no mask stream, no mask
DMA.  Each core's rhs columns are rotated by -core*R so the diagonal band
sits at the same local columns on every core (one SPMD program); block 0
gets an extra wrapped window piece [N-64, N).  Min/max are column-order
invariant so the decode is unaffected.

A custom single-src dual min/max DVE op (ANT_DUAL_MINMAX_SS) computes both
reductions in one pass and carries uop programs for ALL four DVE perf modes;
on fp16 SBUF data it runs at 4 elem/lane/cycle.  Outputs are mode-invariant:
a stride-0 [min,max] pair word (min of the pair = row min in every mode)
plus the fp32 max accumulator (accum_out = row max).

Engine split per core (R=1024 rows x N=8192 cols = 8.4M cells):
  PE : DoubleRow fp8 matmuls + band correction (idle-ish, ~10us)
  ACT: copies S staged PSUM chunks per row-block to SBUF fp16 (1 elem/cyc)
  DVE: 4x-scans the staged fp16 rows + 1x-scans the remaining PSUM chunks
The staged/direct split (S_PATTERN) balances ACT vs DVE at ~42us/core vs the
v3 dual-op floor of ~82us.

Host: recovers labels (np.unique of mask rows), sorts, PCA-rotates (exact for
distances), packs fp8, decodes d2, applies margins/AvgNonZero, and computes
validity (has_pos/has_neg) exactly from the input masks.  If the masks do not
have label structure (never for this problem's generator), falls back to an
exact numpy implementation.
"""

import numpy as np
import ml_dtypes

import concourse.bass as bass  # noqa: F401
import concourse.mybir as mybir
import concourse.tile as tile
from concourse import bacc
from concourse import dve_ops as _dvo
from concourse.bass_utils import run_bass_kernel_spmd
from concourse.dve_spec import C1, MaxNeg, Spec, Src0, maxx, minn
from concourse.dve_table_gen import dve_ver_for
from concourse.dve_uop import (
    DISABLE,
    ENABLE,
    AluInp,
    AluOp,
    DelayInp,
    DveOpSpec,
    InpSel,
    OutPath,
    OutSel,
    Trigger,
    UopConfig,
)

N_CORES = 8
JCH = 2048
SEG = 512
PSUM_BUFS = 2
CAP = 64          # max classes per 128-row block AND max class size (asserted)
WINW = 256        # per-block correction window slot width
B_FILL = 128.0    # same-class bias (16*8, fp8-exact)
A_OFF = 16.0      # constant shift folded into the sq rows
MIN_SEED = 3.0e38
POS_MARGIN = 0.2
NEG_MARGIN = 0.2
# chunks staged via ACT->fp16 per row-block (rest scanned 1x from PSUM).
# NOTE: custom-DVE perf modes (PERF_MAX>0) are firmware-broken on this
# snapshot (T1 incomplete; the engine faults) — staging only pays off with
# >=2x scans, so the shipping config is all-direct 1x PSUM scans.
S_PATTERN = (0, 0, 0, 0, 0, 0, 0, 0)
PERF_MAX = 0  # highest DVE perf-mode slot advertised on staged scans

F32 = mybir.dt.float32
F16 = mybir.dt.float16
FP8 = mybir.dt.float8e4


# --------------------------------------------------------------------------
# Custom DVE op: single-src dual min/max, all four perf modes.
#
# Steady-state semantics (per instruction):
#   out pair word <- [running_min, running_max] every cycle (stride-0 dst);
#     in 1x mode only the lo half is written (running_min), so
#     min(out[0], out[1]) == row min in EVERY mode.
#   accum_out <- row max (fp32, via block-7 a-flop accumulator).
# A 1-cycle seed uop initializes the min recurrence to C1 (s1=+3e38) and the
# max recurrence to MAX_NEG.
# --------------------------------------------------------------------------


def _seed_from(steady: UopConfig, min_blk: int) -> UopConfig:
    """Seed uop: block min_blk's flop <- C1 (via chain4), block 7's flop <-
    MAX_NEG (via chain5)."""
    u = UopConfig()
    u.inp = list(steady.inp)
    u.inp_enable = list(steady.inp_enable)
    dp = u.datapath_config
    for b in range(min_blk):
        dp[b].pass_through_delay(4, 5)
    dp[min_blk].enable_alu(
        AluOp.BYPASS, AluInp.PREV_DELAY_4, AluInp.PREV_DELAY_4
    ).pass_through_delay(5)
    for b in range(min_blk + 1, 7):
        dp[b].pass_through_delay(5)
    dp[7].enable_alu(AluOp.BYPASS, AluInp.PREV_DELAY_5, AluInp.PREV_DELAY_5)
    dp[7].alu_out_a_enable = ENABLE
    for p in OutPath:
        u.out_enable[p] = DISABLE
    u.require_inp0 = DISABLE
    u.require_inp1 = DISABLE
    u.repeat_count = 1
    u.trigger = (Trigger.COUNT, Trigger.NONE, Trigger.NONE)
    u.next_uop = (1, 0, 0)
    u.accum_enabled = ENABLE
    return u


def _steady_common(u: UopConfig):
    u.enable_input(InpSel.CONST_1, 5)   # chain4 seed value (block-0 view only)
    u.enable_input(InpSel.MAX_NEG, 6)   # chain5 seed value
    u.require_inp0 = ENABLE
    u.require_inp1 = DISABLE
    u.trigger = (Trigger.SRC_TENSOR_DONE, Trigger.NONE, Trigger.NONE)
    u.next_uop = (0, 0, 0)
    u.accum_enabled = ENABLE


def _mk_1x() -> list[UopConfig]:
    u = UopConfig()
    u.enable_input(InpSel.SRC_0, 1)  # chain0 = x
    _steady_common(u)
    dp = u.datapath_config
    # b0: min_rec = MIN(self, x); chain4 <- new min; carry x on chain0
    dp[0].enable_alu(
        AluOp.MIN, AluInp.CURR_ALU_OUT, AluInp.PREV_DELAY_0
    ).pass_through_delay(0).enable_delay_from_src(DelayInp.CURR_ALU_OUT, 4)
    for b in range(1, 7):
        dp[b].pass_through_delay(0, 4)
    # b7: max_rec = MAX(self, x); a-flop -> accum
    dp[7].enable_alu(
        AluOp.MAX, AluInp.CURR_ALU_OUT, AluInp.PREV_DELAY_0
    ).pass_through_delay(4)
    dp[7].alu_out_a_enable = ENABLE
    u.enable_output(OutSel.DELAY_4, OutPath.WR0_LO)
    return [_seed_from(u, 0), u]


def _mk_2x(second: InpSel) -> list[UopConfig]:
    u = UopConfig()
    u.enable_input(InpSel.SRC_0, 1)  # chain0 = a
    u.enable_input(second, 2)        # chain1 = b
    _steady_common(u)
    dp = u.datapath_config
    # b0: m = MIN(a, b); carry a,b
    dp[0].enable_alu(
        AluOp.MIN, AluInp.PREV_DELAY_0, AluInp.PREV_DELAY_1
    ).pass_through_delay(0, 1)
    # b1: min_rec = MIN(self, m); chain4 <- new min
    dp[1].enable_alu(
        AluOp.MIN, AluInp.CURR_ALU_OUT, AluInp.PREV_ALU_OUT
    ).pass_through_delay(0, 1).enable_delay_from_src(DelayInp.CURR_ALU_OUT, 4)
    for b in range(2, 6):
        dp[b].pass_through_delay(0, 1, 4)
    # b6: M = MAX(a, b)
    dp[6].enable_alu(
        AluOp.MAX, AluInp.PREV_DELAY_0, AluInp.PREV_DELAY_1
    ).pass_through_delay(4)
    # b7: max_rec = MAX(self, M); a-flop -> accum
    dp[7].enable_alu(
        AluOp.MAX, AluInp.CURR_ALU_OUT, AluInp.PREV_ALU_OUT
    ).pass_through_delay(4)
    dp[7].alu_out_a_enable = ENABLE
    u.enable_output(OutSel.DELAY_4, OutPath.WR0_LO)
    u.enable_output(OutSel.ALU_OUT, OutPath.WR0_HI)
    return [_seed_from(u, 1), u]


def _mk_4x() -> list[UopConfig]:
    u = UopConfig()
    u.enable_input(InpSel.SRC_0, 1)     # chain0 = a
    u.enable_input(InpSel.SRC_0_HI, 2)  # chain1 = b
    u.enable_input(InpSel.SRC_1, 3)     # chain2 = c
    u.enable_input(InpSel.SRC_1_HI, 4)  # chain3 = d
    _steady_common(u)
    dp = u.datapath_config
    # min tree: b0 m01, b1 m23 (capture m01->ch4), b2 mq, b3 min_rec
    dp[0].enable_alu(
        AluOp.MIN, AluInp.PREV_DELAY_0, AluInp.PREV_DELAY_1
    ).pass_through_delay(0, 1, 2, 3)
    dp[1].enable_alu(
        AluOp.MIN, AluInp.PREV_DELAY_2, AluInp.PREV_DELAY_3
    ).pass_through_delay(0, 1, 2, 3).enable_delay_from_src(
        DelayInp.PREV_ALU_OUT, 4
    )
    dp[2].enable_alu(
        AluOp.MIN, AluInp.PREV_ALU_OUT, AluInp.PREV_DELAY_4
    ).pass_through_delay(0, 1, 2, 3)
    dp[3].enable_alu(
        AluOp.MIN, AluInp.CURR_ALU_OUT, AluInp.PREV_ALU_OUT
    ).pass_through_delay(0, 1, 2, 3).enable_delay_from_src(
        DelayInp.CURR_ALU_OUT, 4
    )
    # max tree: b4 M01, b5 M23 (capture M01->ch5), b6 Mq, b7 max_rec
    dp[4].enable_alu(
        AluOp.MAX, AluInp.PREV_DELAY_0, AluInp.PREV_DELAY_1
    ).pass_through_delay(2, 3, 4)
    dp[5].enable_alu(
        AluOp.MAX, AluInp.PREV_DELAY_2, AluInp.PREV_DELAY_3
    ).pass_through_delay(4).enable_delay_from_src(DelayInp.PREV_ALU_OUT, 5)
    dp[6].enable_alu(
        AluOp.MAX, AluInp.PREV_ALU_OUT, AluInp.PREV_DELAY_5
    ).pass_through_delay(4)
    dp[7].enable_alu(
        AluOp.MAX, AluInp.CURR_ALU_OUT, AluInp.PREV_ALU_OUT
    ).pass_through_delay(4)
    dp[7].alu_out_a_enable = ENABLE
    u.enable_output(OutSel.DELAY_4, OutPath.WR0_LO)
    u.enable_output(OutSel.ALU_OUT, OutPath.WR0_HI)
    u.enable_output(OutSel.DELAY_4, OutPath.WR1_LO)
    u.enable_output(OutSel.ALU_OUT, OutPath.WR1_HI)
    return [_seed_from(u, 3), u]


def _minmax_reference(in0, in1, c0, c1, c2):
    """CoreSim reference (1x semantics).  out = running min (the stride-0
    dst keeps only the final pair; min(lo,hi)=row min in every mode);
    accum = row max."""
    x = np.asarray(in0, np.float32)
    x2 = x.reshape(x.shape[0], -1)
    c1r = np.asarray(c1, np.float32).reshape(-1, 1)
    run = np.minimum.accumulate(np.minimum(x2, c1r), axis=-1)
    acc = np.max(x2, axis=-1, keepdims=True)
    return run, acc


def _register_minmax_op(name: str = "ANT_DUAL_MINMAX_SS"):
    for op in _dvo.OPS:
        if op.name == name:
            return op
    row = _dvo._CUSTOM_DVE_ROW_BASE + len(_dvo.OPS)
    assert row < 0x20, "custom-DVE sub-opcode rows exhausted"
    _dvo._SUB_OPCODE_FOR_NAME[name] = row
    ver = dve_ver_for("TRN2")
    u1 = _mk_1x()
    u2 = _mk_2x(InpSel.SRC_0_HI)
    u2p = _mk_2x(InpSel.SRC_1)
    u4 = _mk_4x()
    for us in (u1, u2, u2p, u4):
        for u in us:
            u.validate(ver)
    opspec = DveOpSpec(
        name=name,
        opcode=row,
        uops=u1,
        uops_2x=u2,
        uops_2x_2p=u2p,
        uops_4x=u4,
        perf_max=3,
        rd1_en=False,
    )
    sha = opspec.sha(ver)
    carrier = Spec(
        body=minn(Src0, C1),
        accum=maxx,
        accum_init=MaxNeg,
        reference=_minmax_reference,
    )
    op = _dvo.DveOp(
        name, carrier, subdim=False, uops_sha={ver: sha}, perf_en={ver: True}
    )
    _dvo._COMPILE_CACHE[(name, ver)] = opspec
    _dvo.OPS.append(op)
    _dvo.CUSTOM_DVE_SPECS[name] = carrier
    return op


MINMAX_OP = _register_minmax_op()


# --------------------------------------------------------------------------
# Schedules shared by trace, host prep, and decode (all in the per-core
# LOCAL column space: core ci sees global sorted columns rolled by -ci*R).
# --------------------------------------------------------------------------


def _corr_pieces_local(ib: int, n: int):
    """Correction pieces for local row-block ib as (slot_off, a, b): local
    columns [a, b) mapped from rcorr slot columns [slot_off, slot_off+b-a).
    Pieces never cross a 512-col bank boundary.  Block 0 wraps: its window
    is [0, 192) plus the wrapped tail [n-64, n)."""
    if ib == 0:
        return [(0, 0, 192), (192, n - 64, n)]
    w0 = ib * 128 - 64
    w1 = w0 + WINW
    pieces = []
    a = w0
    while a < w1:
        b = min(w1, (a // SEG + 1) * SEG)
        pieces.append((a - w0, a, b))
        a = b
    return pieces


def _op_schedule(n_ib: int, n_jc: int, s_pattern):
    """[(ib, jc_lo, jc_hi, kind)] in DVE trace order."""
    ops = []
    for ib in range(n_ib):
        s = s_pattern[ib % len(s_pattern)]
        if s > 0:
            ops.append((ib, 0, s, "run"))
        for jc in range(s, n_jc):
            ops.append((ib, jc, jc + 1, "direct"))
    return ops


# --------------------------------------------------------------------------
# Device program (identical for every core)
# --------------------------------------------------------------------------


def build_nc(R, N, D, s_pattern=S_PATTERN, repeat=1, bench_skip=()):
    assert R % 128 == 0 and N % JCH == 0 and D == 256
    n_ib = R // 128
    n_jc = N // JCH
    n_seg = JCH // SEG
    ops = _op_schedule(n_ib, n_jc, s_pattern)
    n_ops = len(ops)
    smax = max(s_pattern)

    nc = bacc.Bacc(None, target_bir_lowering=False)
    lhs_d = nc.dram_tensor("lhs8", [128, 2, R], FP8, kind="ExternalInput")
    rhs_d = nc.dram_tensor("rhs8", [128, 2, N], FP8, kind="ExternalInput")
    lcorr_d = nc.dram_tensor("lcorr", [CAP, n_ib * 128], FP8, kind="ExternalInput")
    rcorr_d = nc.dram_tensor("rcorr", [CAP, n_ib * WINW], FP8, kind="ExternalInput")
    res_d = nc.dram_tensor("res", [128, n_ops, 2], F16, kind="ExternalOutput")
    acc_d = nc.dram_tensor("acc", [128, n_ops], F32, kind="ExternalOutput")

    with tile.TileContext(nc) as tc:
        with (
            tc.tile_pool(name="const", bufs=1) as cpool,
            tc.tile_pool(name="stage", bufs=2) as spool,
            tc.tile_pool(name="psum", bufs=PSUM_BUFS, space="PSUM") as ppool,
            tc.tile_pool(name="out", bufs=1) as opool,
        ):
            rhs_sb = cpool.tile([128, 2, N], FP8, tag="rhs", name="rhs_sb")
            lhs_sb = cpool.tile([128, 2, R], FP8, tag="lhs", name="lhs_sb")
            lcorr_sb = cpool.tile([CAP, n_ib * 128], FP8, tag="lc", name="lcorr_sb")
            rcorr_sb = cpool.tile([CAP, n_ib * WINW], FP8, tag="rc", name="rcorr_sb")
            nc.sync.dma_start(rhs_sb[:], rhs_d[:])
            nc.sync.dma_start(lhs_sb[:], lhs_d[:])
            nc.sync.dma_start(lcorr_sb[:], lcorr_d[:])
            nc.sync.dma_start(rcorr_sb[:], rcorr_d[:])

            res = opool.tile([128, n_ops, 2], F16, tag="res", name="res")
            acc = opool.tile([128, n_ops], F32, tag="acc", name="acc")

            def scan(op_id, in0_ap, n_elem, staged):
                if staged and PERF_MAX > 0:
                    # dense dst: write counts are mode-invariant (K writes in
                    # 1x/2x/4x), unlike the stride-0 broadcast dst.  The tail
                    # pair [K-2:K] is [run_min, run_max] in packed modes and
                    # [run_min(K-2), run_min(K-1)] at 1x; min(pair) = row min
                    # either way.  Row max comes from the accumulator.
                    sc = spool.tile(
                        [128, smax * JCH], F16, tag="sc", name=f"sc{op_id}"
                    )
                    r = nc.vector._custom_dve(
                        MINMAX_OP,
                        out=sc[:, :n_elem],
                        in0=in0_ap,
                        s1=MIN_SEED,
                        accum_out=acc[:, op_id : op_id + 1],
                    )
                    r.ins.perf_max = PERF_MAX
                    nc.sync.dma_start(
                        res[:, op_id, :], sc[:, n_elem - 2 : n_elem]
                    )
                    return r
                r = nc.vector._custom_dve(
                    MINMAX_OP,
                    out=res[:, op_id : op_id + 1, :].broadcast_to(
                        (128, n_elem // 2, 2)
                    ),
                    in0=in0_ap,
                    s1=MIN_SEED,
                    accum_out=acc[:, op_id : op_id + 1],
                )
                # direct PSUM fp32 scans are REGULAR-only — don't advertise
                # perf modes on them.
                r.ins.perf_max = 0
                return r

            def trace_body():
                op_id = 0
                for ib in range(n_ib):
                    s = s_pattern[ib % len(s_pattern)]
                    pieces = _corr_pieces_local(ib, N)
                    ibsl = slice(ib * 128, (ib + 1) * 128)
                    st = (
                        spool.tile(
                            [128, smax * JCH], F16, tag="st", name=f"st{ib}"
                        )
                        if s > 0
                        else None
                    )
                    run_op = op_id if s > 0 else None
                    direct_base = op_id + (1 if s > 0 else 0)
                    for jc in range(n_jc):
                        pt = ppool.tile(
                            [128, JCH], F32, tag="pt", name=f"pt{ib}_{jc}"
                        )
                        cj0 = jc * JCH
                        my_pieces = [
                            (so, a, b)
                            for (so, a, b) in pieces
                            if cj0 <= a < cj0 + JCH
                        ]
                        corr_banks = {a // SEG for (_, a, b) in my_pieces}
                        if "pe" not in bench_skip:
                            for sg in range(n_seg):
                                j0 = cj0 + sg * SEG
                                nc.tensor.matmul(
                                    pt[:, sg * SEG : (sg + 1) * SEG],
                                    lhs_sb[:, :, ibsl],
                                    rhs_sb[:, :, j0 : j0 + SEG],
                                    start=True,
                                    stop=(j0 // SEG) not in corr_banks,
                                    perf_mode=mybir.MatmulPerfMode.DoubleRow,
                                    skip_group_check=True,
                                )
                            for so, a, b in my_pieces:
                                nc.tensor.matmul(
                                    pt[:, a - cj0 : b - cj0],
                                    lcorr_sb[:, ibsl],
                                    rcorr_sb[
                                        :, ib * WINW + so : ib * WINW + so + (b - a)
                                    ],
                                    start=False,
                                    stop=True,
                                    skip_group_check=True,
                                )
                        else:
                            nc.scalar.memzero(pt[:])
                        if jc < s:
                            if "act" not in bench_skip:
                                nc.scalar.copy(
                                    st[:, jc * JCH : (jc + 1) * JCH], pt[:]
                                )
                            if jc == s - 1 and "dve" not in bench_skip:
                                scan(run_op, st[:, : s * JCH], s * JCH, True)
                        else:
                            if "dve" not in bench_skip:
                                scan(
                                    direct_base + (jc - s), pt[:], JCH, False
                                )
                    op_id = direct_base + (n_jc - s)

            if repeat == 1:
                trace_body()
            else:
                with tc.For_i(0, repeat, 1):
                    trace_body()
            if "dve" not in bench_skip:
                nc.sync.dma_start(res_d[:], res[:])
                nc.sync.dma_start(acc_d[:], acc[:])
    nc.compile()
    return nc


# --------------------------------------------------------------------------
# Host side
# --------------------------------------------------------------------------


def _avg_nonzero(losses):
    nz = np.count_nonzero(losses > 0)
    return float(np.sum(losses) / nz) if nz > 0 else 0.0


def _pack_fp8(a2d):
    """[256, M] -> DoubleRow-packed [128, 2, M] fp8e4m3."""
    d, m = a2d.shape
    assert d == 256
    return np.ascontiguousarray(
        a2d.reshape(2, 128, m).transpose(1, 0, 2)
    ).astype(ml_dtypes.float8_e4m3)


def _extract_labels(pos, neg):
    """Recover labels from the masks; None if they lack label structure."""
    packed = np.packbits(neg, axis=1)
    key = packed.view([("", f"V{packed.shape[1]}")]).ravel()
    _, labels = np.unique(key, return_inverse=True)
    same = labels[:, None] == labels[None, :]
    if np.array_equal(neg, ~same):
        np.fill_diagonal(same, False)
        if np.array_equal(pos, same):
            return labels
    return None


def _host_reference(x, pos, neg):
    """Exact numpy fallback for non-label-structured masks."""
    x = np.asarray(x, np.float32)
    sq = np.sum(x * x, axis=1)
    d2 = sq[:, None] + sq[None, :] - 2.0 * (x @ x.T)
    dist = np.sqrt(np.maximum(d2, 1e-12), dtype=np.float32)
    has_pos = pos.any(axis=1)
    has_neg = neg.any(axis=1)
    valid = has_pos & has_neg
    hp = np.max(np.where(pos, dist, -1.0), axis=1)
    hn = np.min(np.where(neg, dist, 1e10), axis=1)
    pl = np.where(valid, np.maximum(hp - POS_MARGIN, 0.0), 0.0)
    nl = np.where(valid, np.maximum(NEG_MARGIN - hn, 0.0), 0.0)
    return np.float32(_avg_nonzero(pl) + _avg_nonzero(nl))


def _prep_inputs(embeddings, positives_mask, negatives_mask, n_cores):
    x = np.asarray(embeddings, dtype=np.float32)
    pos = np.asarray(positives_mask).astype(bool)
    neg = np.asarray(negatives_mask).astype(bool)
    n, d = x.shape
    r = n // n_cores

    labels = _extract_labels(pos, neg)
    if labels is None:
        return None, {"fallback": True, "x": x, "pos": pos, "neg": neg}

    perm = np.argsort(labels, kind="stable")
    labels_s = labels[perm]
    starts = np.flatnonzero(np.r_[True, labels_s[1:] != labels_s[:-1]])
    sizes = np.diff(np.r_[starts, n])
    if sizes.max() > CAP:
        return None, {"fallback": True, "x": x, "pos": pos, "neg": neg}
    cls_of_row = np.repeat(np.arange(len(starts)), sizes)

    x64 = x.astype(np.float64)[perm]
    sq = np.sum(x64**2, axis=1)
    sq_max, sq_min = float(sq.max()), float(sq.min())
    s = 0.125
    while 5.0 * s * sq_max - s * sq_min > B_FILL - 14.0 and s > 2.0**-40:
        s *= 0.5

    _, V = np.linalg.eigh(x64.T @ x64)
    xr = x64 @ V[:, 2:]  # [N, 254]; rotation preserves distances
    f8 = ml_dtypes.float8_e4m3
    c = np.sqrt(2.0 * s)
    sqs = (s * sq - A_OFF).astype(np.float32)
    sq_hi = sqs.astype(f8)
    sq_lo = (sqs - sq_hi.astype(np.float32)).astype(f8)

    rhs_aug = np.empty((d, n), dtype=np.float32)
    rhs_aug[: d - 2] = (c * xr.T).astype(f8).astype(np.float32)
    rhs_aug[d - 2] = sq_hi.astype(np.float32)
    rhs_aug[d - 1] = sq_lo.astype(np.float32)
    rhs_full = _pack_fp8(rhs_aug)  # [128, 2, N] in global sorted col order
    lhs_aug_full = np.empty((d, n), dtype=np.float32)
    lhs_aug_full[: d - 2] = (-c * xr.T).astype(f8).astype(np.float32)
    lhs_aug_full[d - 2 :] = 1.0

    n_ib = r // 128
    in_maps = []
    for ci in range(n_cores):
        rows = slice(ci * r, (ci + 1) * r)
        lhs = _pack_fp8(np.ascontiguousarray(lhs_aug_full[:, rows]))
        rhs_ci = np.ascontiguousarray(np.roll(rhs_full, -ci * r, axis=2))
        lcorr = np.zeros((CAP, n_ib * 128), dtype=f8)
        rcorr = np.zeros((CAP, n_ib * WINW), dtype=f8)
        ok = True
        for ib in range(n_ib):
            g0 = (ci * n_ib + ib) * 128
            local = np.unique(cls_of_row[g0 : g0 + 128])
            if len(local) > CAP:
                ok = False
                break
            lmap = np.full(cls_of_row.max() + 1, -1, dtype=np.int64)
            lmap[local] = np.arange(len(local))
            lcorr[lmap[cls_of_row[g0 : g0 + 128]], ib * 128 + np.arange(128)] = 16.0
            for so, a, b in _corr_pieces_local(ib, n):
                gcols = (np.arange(a, b) + ci * r) % n
                cls = cls_of_row[gcols]
                sel = np.flatnonzero(lmap[cls] >= 0)
                rcorr[lmap[cls[sel]], ib * WINW + so + sel] = 8.0
        if not ok:
            return None, {"fallback": True, "x": x, "pos": pos, "neg": neg}
        in_maps.append(
            {"lhs8": lhs, "rhs8": rhs_ci, "lcorr": lcorr, "rcorr": rcorr}
        )
    aux = {
        "fallback": False,
        "sq": sq,
        "s": s,
        "perm": perm,
        "n": n,
        "r": r,
        "has_pos": pos.any(axis=1),
        "has_neg": neg.any(axis=1),
    }
    return in_maps, aux


def _decode(results, aux, n_cores, s_pattern=S_PATTERN):
    sq, s, perm = aux["sq"], aux["s"], aux["perm"]
    n, r = aux["n"], aux["r"]
    n_ib = r // 128
    n_jc = n // JCH
    ops = _op_schedule(n_ib, n_jc, s_pattern)

    rmin = np.full(n, np.inf)
    rmax = np.full(n, -np.inf)
    for ci in range(n_cores):
        res = np.asarray(results[ci]["res"], dtype=np.float64)  # [128, n_ops, 2]
        acc = np.asarray(results[ci]["acc"], dtype=np.float64)  # [128, n_ops]
        pair_min = np.minimum(res[:, :, 0], res[:, :, 1])
        for k, (ib, _, _, _) in enumerate(ops):
            rows = slice(ci * r + ib * 128, ci * r + (ib + 1) * 128)
            rmin[rows] = np.minimum(rmin[rows], pair_min[:, k])
            rmax[rows] = np.maximum(rmax[rows], acc[:, k])

    pos_d2 = (rmax - B_FILL + A_OFF) / s + sq
    neg_d2 = (rmin + A_OFF) / s + sq
    hp = np.sqrt(np.maximum(pos_d2, 1e-12))
    hn = np.sqrt(np.maximum(neg_d2, 1e-12))
    valid = (aux["has_pos"] & aux["has_neg"])[perm]
    pos_loss = np.where(valid, np.maximum(hp - POS_MARGIN, 0.0), 0.0)
    neg_loss = np.where(valid, np.maximum(NEG_MARGIN - hn, 0.0), 0.0)
    return np.float32(_avg_nonzero(pos_loss) + _avg_nonzero(neg_loss))


_NC_CACHE = {}


def _kernel_impl(embeddings, positives_mask, negatives_mask, trace=False):
    x = np.asarray(embeddings)
    n, d = x.shape
    in_maps, aux = _prep_inputs(
        embeddings, positives_mask, negatives_mask, N_CORES
    )
    if aux.get("fallback"):
        return _host_reference(aux["x"], aux["pos"], aux["neg"]), None
    key = (n // N_CORES, n, d)
    if key not in _NC_CACHE:
        _NC_CACHE[key] = build_nc(*key)
    nc = _NC_CACHE[key]
    out = run_bass_kernel_spmd(nc, in_maps, list(range(N_CORES)), trace=trace)
    result = _decode(out.results, aux, N_CORES)
    return result, out


def kernel(embeddings, positives_mask, negatives_mask):
    result, _ = _kernel_impl(embeddings, positives_mask, negatives_mask)
    return result


# revision 17
# speedup vs baseline: 1.0014x; 1.0014x over previous
"""BatchHardContrastiveLoss Trainium2 kernel (8-core SPMD), v4.

Math: with rows/columns class-sorted (labels recovered on the host from the
mask structure), the PE emits ONE biased matrix per anchor tile:

    v[i,j] = s*(d2(i,j) - sq_i) - A + B*same[i,j]

The B*same band (classes are contiguous after sorting, so same(i,j) for a
128-row block lives in a fixed 256-wide diagonal window) is added by a tiny
rank-<=64 one-hot fp8 matmul accumulated onto the main DoubleRow matmul's
PSUM segments.  B > 5*s*sq_max guarantees strict separation, so

    row max of v  = B - A + s*(hardest_pos_d2 - sq_i)   (pos mining, diag incl.)
    row min of v  =   - A + s*(hardest_neg_d2 - sq_i)   (neg mining)

i.e. both reductions are PLAIN unmasked max/min — no mask stream, no mask
DMA (v3 DMA'd a 64 MB u8 mask plane and fed it to the DVE as a second
operand).  Each core's rhs columns are rotated by -core*R so the diagonal
band sits at the same local columns on every core (one SPMD program); block
0 gets an extra wrapped window piece [N-64, N).  Min/max are column-order
invariant so the decode is unaffected.

A custom single-src dual min/max DVE op (ANT_DUAL_MINMAX_SS) computes both
reductions in one 1x pass per PSUM chunk: a stride-0 [min,max] pair word
(min of the pair = row min) plus the fp32 max accumulator (accum_out = row
max).  The op also carries hand-written uop programs for the 2x/4x DVE perf
modes, but the custom-DVE perf-mode dispatch is firmware-broken on this
snapshot (the T1 parity test is xfail; any perf_max>0 faults the engine,
verified down to a trivial delay-only op), so they are never advertised
(PERF_MAX=0).  With modes dead, every engine path bottoms out at the same
1 cell/lane/cycle DVE wall (PSUM port = 32b/cyc; stock TT/reduce trees
re-read intermediates; ACT accumulates sums only; GpSimd has no PSUM port
and its tensor_tensor fails to compile), so the kernel scans all 8.4M
cells/core on the DVE at 1x: 65536 cyc @ 0.96 GHz = 68.3us floor, ~72-74us
measured (v3 baseline: 81.8us; the win is the removed mask stream and
leaner per-op overhead).

Engine roles per core (R=1024 rows x N=8192 cols):
  PE : DoubleRow fp8 matmuls + rank-<=64 band correction (~15% busy)
  DVE: dual min/max scan of each [128, 2048] PSUM chunk (bottleneck)
  ACT/GpSimd: idle ("row"/"gp" BLOCK_MODES exist but don't pay off at 1x)

Host: recovers labels (np.unique of mask rows), sorts, PCA-rotates (exact
for distances), packs fp8, decodes d2, applies margins/AvgNonZero, and
computes validity (has_pos/has_neg) exactly from the input masks.  If the
masks do not have label structure (never for this problem's generator),
falls back to an exact numpy implementation.
"""

import numpy as np
import ml_dtypes

import concourse.bass as bass  # noqa: F401
import concourse.mybir as mybir
import concourse.tile as tile
from concourse import bacc
from concourse import dve_ops as _dvo
from concourse.bass_utils import run_bass_kernel_spmd
from concourse.dve_spec import C1, MaxNeg, Spec, Src0, maxx, minn
from concourse.dve_table_gen import dve_ver_for
from concourse.dve_uop import (
    DISABLE,
    ENABLE,
    AluInp,
    AluOp,
    DelayInp,
    DveOpSpec,
    InpSel,
    OutPath,
    OutSel,
    Trigger,
    UopConfig,
)

N_CORES = 8
JCH = 2048
SEG = 512
PSUM_BUFS = 2
CAP = 64          # max classes per 128-row block AND max class size (asserted)
WINW = 256        # per-block correction window slot width
B_FILL = 128.0    # same-class bias (16*8, fp8-exact)
A_OFF = 16.0      # constant shift folded into the sq rows
MIN_SEED = 3.0e38
POS_MARGIN = 0.2
NEG_MARGIN = 0.2
# NOTE: custom-DVE perf modes (PERF_MAX>0) are firmware-broken on this
# snapshot (T1 incomplete; the engine faults on any mode >= 2x_1p), so all
# DVE scans run REGULAR 1x.  Per-block modes:
#   ("direct", 0): DVE dual-scans the 4 PSUM chunks directly (no copies)
#   ("row", 4):    ACT copies all 4 chunks to fp16 SBUF; DVE scans the full
#                  row in ONE op (amortizes the per-op drain)
#   ("gp", s):     ACT copies s chunks to fp16 SBUF; GpSimd runs pairwise
#                  min/max trees; DVE scans the <=512-wide tails; the other
#                  4-s chunks go direct
# Measured on HW: all-direct ~72-74us, all-row ~75us (statistically tied,
# both near the 68.3us 1x-scan floor); "gp" does not compile (walrus rejects
# nc.gpsimd.tensor_tensor in this toolchain).  Ship all-direct: fewest
# engines, no staging dependencies.
BLOCK_MODES = (("direct", 0),) * 8
PERF_MAX = 0  # custom-DVE perf modes are firmware-broken; keep 0

F32 = mybir.dt.float32
F16 = mybir.dt.float16
FP8 = mybir.dt.float8e4


# --------------------------------------------------------------------------
# Custom DVE op: single-src dual min/max, all four perf modes.
#
# Steady-state semantics (per instruction):
#   out pair word <- [running_min, running_max] every cycle (stride-0 dst);
#     in 1x mode only the lo half is written (running_min), so
#     min(out[0], out[1]) == row min in EVERY mode.
#   accum_out <- row max (fp32, via block-7 a-flop accumulator).
# A 1-cycle seed uop initializes the min recurrence to C1 (s1=+3e38) and the
# max recurrence to MAX_NEG.
# --------------------------------------------------------------------------


def _seed_from(steady: UopConfig, min_blk: int) -> UopConfig:
    """Seed uop: block min_blk's flop <- C1 (via chain4), block 7's flop <-
    MAX_NEG (via chain5)."""
    u = UopConfig()
    u.inp = list(steady.inp)
    u.inp_enable = list(steady.inp_enable)
    dp = u.datapath_config
    for b in range(min_blk):
        dp[b].pass_through_delay(4, 5)
    dp[min_blk].enable_alu(
        AluOp.BYPASS, AluInp.PREV_DELAY_4, AluInp.PREV_DELAY_4
    ).pass_through_delay(5)
    for b in range(min_blk + 1, 7):
        dp[b].pass_through_delay(5)
    dp[7].enable_alu(AluOp.BYPASS, AluInp.PREV_DELAY_5, AluInp.PREV_DELAY_5)
    dp[7].alu_out_a_enable = ENABLE
    for p in OutPath:
        u.out_enable[p] = DISABLE
    u.require_inp0 = DISABLE
    u.require_inp1 = DISABLE
    u.repeat_count = 1
    u.trigger = (Trigger.COUNT, Trigger.NONE, Trigger.NONE)
    u.next_uop = (1, 0, 0)
    u.accum_enabled = ENABLE
    return u


def _steady_common(u: UopConfig):
    u.enable_input(InpSel.CONST_1, 5)   # chain4 seed value (block-0 view only)
    u.enable_input(InpSel.MAX_NEG, 6)   # chain5 seed value
    u.require_inp0 = ENABLE
    u.require_inp1 = DISABLE
    u.trigger = (Trigger.SRC_TENSOR_DONE, Trigger.NONE, Trigger.NONE)
    u.next_uop = (0, 0, 0)
    u.accum_enabled = ENABLE


def _mk_1x() -> list[UopConfig]:
    u = UopConfig()
    u.enable_input(InpSel.SRC_0, 1)  # chain0 = x
    _steady_common(u)
    dp = u.datapath_config
    # b0: min_rec = MIN(self, x); chain4 <- new min; carry x on chain0
    dp[0].enable_alu(
        AluOp.MIN, AluInp.CURR_ALU_OUT, AluInp.PREV_DELAY_0
    ).pass_through_delay(0).enable_delay_from_src(DelayInp.CURR_ALU_OUT, 4)
    for b in range(1, 7):
        dp[b].pass_through_delay(0, 4)
    # b7: max_rec = MAX(self, x); a-flop -> accum
    dp[7].enable_alu(
        AluOp.MAX, AluInp.CURR_ALU_OUT, AluInp.PREV_DELAY_0
    ).pass_through_delay(4)
    dp[7].alu_out_a_enable = ENABLE
    u.enable_output(OutSel.DELAY_4, OutPath.WR0_LO)
    return [_seed_from(u, 0), u]


def _mk_2x(second: InpSel) -> list[UopConfig]:
    u = UopConfig()
    u.enable_input(InpSel.SRC_0, 1)  # chain0 = a
    u.enable_input(second, 2)        # chain1 = b
    _steady_common(u)
    dp = u.datapath_config
    # b0: m = MIN(a, b); carry a,b
    dp[0].enable_alu(
        AluOp.MIN, AluInp.PREV_DELAY_0, AluInp.PREV_DELAY_1
    ).pass_through_delay(0, 1)
    # b1: min_rec = MIN(self, m); chain4 <- new min
    dp[1].enable_alu(
        AluOp.MIN, AluInp.CURR_ALU_OUT, AluInp.PREV_ALU_OUT
    ).pass_through_delay(0, 1).enable_delay_from_src(DelayInp.CURR_ALU_OUT, 4)
    for b in range(2, 6):
        dp[b].pass_through_delay(0, 1, 4)
    # b6: M = MAX(a, b)
    dp[6].enable_alu(
        AluOp.MAX, AluInp.PREV_DELAY_0, AluInp.PREV_DELAY_1
    ).pass_through_delay(4)
    # b7: max_rec = MAX(self, M); a-flop -> accum
    dp[7].enable_alu(
        AluOp.MAX, AluInp.CURR_ALU_OUT, AluInp.PREV_ALU_OUT
    ).pass_through_delay(4)
    dp[7].alu_out_a_enable = ENABLE
    u.enable_output(OutSel.DELAY_4, OutPath.WR0_LO)
    u.enable_output(OutSel.ALU_OUT, OutPath.WR0_HI)
    return [_seed_from(u, 1), u]


def _mk_4x() -> list[UopConfig]:
    u = UopConfig()
    u.enable_input(InpSel.SRC_0, 1)     # chain0 = a
    u.enable_input(InpSel.SRC_0_HI, 2)  # chain1 = b
    u.enable_input(InpSel.SRC_1, 3)     # chain2 = c
    u.enable_input(InpSel.SRC_1_HI, 4)  # chain3 = d
    _steady_common(u)
    dp = u.datapath_config
    # min tree: b0 m01, b1 m23 (capture m01->ch4), b2 mq, b3 min_rec
    dp[0].enable_alu(
        AluOp.MIN, AluInp.PREV_DELAY_0, AluInp.PREV_DELAY_1
    ).pass_through_delay(0, 1, 2, 3)
    dp[1].enable_alu(
        AluOp.MIN, AluInp.PREV_DELAY_2, AluInp.PREV_DELAY_3
    ).pass_through_delay(0, 1, 2, 3).enable_delay_from_src(
        DelayInp.PREV_ALU_OUT, 4
    )
    dp[2].enable_alu(
        AluOp.MIN, AluInp.PREV_ALU_OUT, AluInp.PREV_DELAY_4
    ).pass_through_delay(0, 1, 2, 3)
    dp[3].enable_alu(
        AluOp.MIN, AluInp.CURR_ALU_OUT, AluInp.PREV_ALU_OUT
    ).pass_through_delay(0, 1, 2, 3).enable_delay_from_src(
        DelayInp.CURR_ALU_OUT, 4
    )
    # max tree: b4 M01, b5 M23 (capture M01->ch5), b6 Mq, b7 max_rec
    dp[4].enable_alu(
        AluOp.MAX, AluInp.PREV_DELAY_0, AluInp.PREV_DELAY_1
    ).pass_through_delay(2, 3, 4)
    dp[5].enable_alu(
        AluOp.MAX, AluInp.PREV_DELAY_2, AluInp.PREV_DELAY_3
    ).pass_through_delay(4).enable_delay_from_src(DelayInp.PREV_ALU_OUT, 5)
    dp[6].enable_alu(
        AluOp.MAX, AluInp.PREV_ALU_OUT, AluInp.PREV_DELAY_5
    ).pass_through_delay(4)
    dp[7].enable_alu(
        AluOp.MAX, AluInp.CURR_ALU_OUT, AluInp.PREV_ALU_OUT
    ).pass_through_delay(4)
    dp[7].alu_out_a_enable = ENABLE
    u.enable_output(OutSel.DELAY_4, OutPath.WR0_LO)
    u.enable_output(OutSel.ALU_OUT, OutPath.WR0_HI)
    u.enable_output(OutSel.DELAY_4, OutPath.WR1_LO)
    u.enable_output(OutSel.ALU_OUT, OutPath.WR1_HI)
    return [_seed_from(u, 3), u]


def _minmax_reference(in0, in1, c0, c1, c2):
    """CoreSim reference (1x semantics).  out = running min (the stride-0
    dst keeps only the final pair; min(lo,hi)=row min in every mode);
    accum = row max."""
    x = np.asarray(in0, np.float32)
    x2 = x.reshape(x.shape[0], -1)
    c1r = np.asarray(c1, np.float32).reshape(-1, 1)
    run = np.minimum.accumulate(np.minimum(x2, c1r), axis=-1)
    acc = np.max(x2, axis=-1, keepdims=True)
    return run, acc


def _register_minmax_op(name: str = "ANT_DUAL_MINMAX_SS"):
    for op in _dvo.OPS:
        if op.name == name:
            return op
    row = _dvo._CUSTOM_DVE_ROW_BASE + len(_dvo.OPS)
    assert row < 0x20, "custom-DVE sub-opcode rows exhausted"
    _dvo._SUB_OPCODE_FOR_NAME[name] = row
    ver = dve_ver_for("TRN2")
    u1 = _mk_1x()
    u2 = _mk_2x(InpSel.SRC_0_HI)
    u2p = _mk_2x(InpSel.SRC_1)
    u4 = _mk_4x()
    for us in (u1, u2, u2p, u4):
        for u in us:
            u.validate(ver)
    opspec = DveOpSpec(
        name=name,
        opcode=row,
        uops=u1,
        uops_2x=u2,
        uops_2x_2p=u2p,
        uops_4x=u4,
        perf_max=3,
        rd1_en=False,
    )
    sha = opspec.sha(ver)
    carrier = Spec(
        body=minn(Src0, C1),
        accum=maxx,
        accum_init=MaxNeg,
        reference=_minmax_reference,
    )
    op = _dvo.DveOp(
        name, carrier, subdim=False, uops_sha={ver: sha}, perf_en={ver: True}
    )
    _dvo._COMPILE_CACHE[(name, ver)] = opspec
    _dvo.OPS.append(op)
    _dvo.CUSTOM_DVE_SPECS[name] = carrier
    return op


MINMAX_OP = _register_minmax_op()


# --------------------------------------------------------------------------
# Schedules shared by trace, host prep, and decode (all in the per-core
# LOCAL column space: core ci sees global sorted columns rolled by -ci*R).
# --------------------------------------------------------------------------


def _corr_pieces_local(ib: int, n: int):
    """Correction pieces for local row-block ib as (slot_off, a, b): local
    columns [a, b) mapped from rcorr slot columns [slot_off, slot_off+b-a).
    Pieces never cross a 512-col bank boundary.  Block 0 wraps: its window
    is [0, 192) plus the wrapped tail [n-64, n)."""
    if ib == 0:
        return [(0, 0, 192), (192, n - 64, n)]
    w0 = ib * 128 - 64
    w1 = w0 + WINW
    pieces = []
    a = w0
    while a < w1:
        b = min(w1, (a // SEG + 1) * SEG)
        pieces.append((a - w0, a, b))
        a = b
    return pieces


GP_TAIL = 512  # GpSimd trees halve down to this width; DVE scans the tail


def _op_schedule(n_ib: int, n_jc: int, modes):
    """Per-block result columns: [(ib, kind)] with kind in
    "both" (pair->min AND acc->max), "min" (pair only), "max" (acc only)."""
    ops = []
    for ib in range(n_ib):
        mode, s = modes[ib % len(modes)]
        if mode == "direct":
            ops += [(ib, "both")] * n_jc
        elif mode == "row":
            ops.append((ib, "both"))
        else:  # gp
            ops += [(ib, "both")] * (n_jc - s)  # direct chunks
            ops.append((ib, "min"))
            ops.append((ib, "max"))
    return ops


# --------------------------------------------------------------------------
# Device program (identical for every core)
# --------------------------------------------------------------------------


def build_nc(R, N, D, modes=BLOCK_MODES, repeat=1, bench_skip=()):
    assert R % 128 == 0 and N % JCH == 0 and D == 256
    n_ib = R // 128
    n_jc = N // JCH
    n_seg = JCH // SEG
    ops = _op_schedule(n_ib, n_jc, modes)
    n_ops = len(ops)
    # op-id bases per block, mirroring _op_schedule's emission order
    op_base = {}
    k = 0
    for ib in range(n_ib):
        op_base[ib] = k
        mode, s = modes[ib % len(modes)]
        k += n_jc if mode == "direct" else (1 if mode == "row" else (n_jc - s) + 2)
    gp_blocks = [ib for ib in range(n_ib) if modes[ib % len(modes)][0] == "gp"]
    # trace order: gp first (GpSimd starts early), then direct (DVE starts
    # early), then row blocks
    trace_order = (
        gp_blocks
        + [ib for ib in range(n_ib) if modes[ib % len(modes)][0] == "direct"]
        + [ib for ib in range(n_ib) if modes[ib % len(modes)][0] == "row"]
    )

    nc = bacc.Bacc(None, target_bir_lowering=False)
    lhs_d = nc.dram_tensor("lhs8", [128, 2, R], FP8, kind="ExternalInput")
    rhs_d = nc.dram_tensor("rhs8", [128, 2, N], FP8, kind="ExternalInput")
    lcorr_d = nc.dram_tensor("lcorr", [CAP, n_ib * 128], FP8, kind="ExternalInput")
    rcorr_d = nc.dram_tensor("rcorr", [CAP, n_ib * WINW], FP8, kind="ExternalInput")
    res_d = nc.dram_tensor("res", [128, n_ops, 2], F16, kind="ExternalOutput")
    acc_d = nc.dram_tensor("acc", [128, n_ops], F32, kind="ExternalOutput")

    with tile.TileContext(nc) as tc:
        with (
            tc.tile_pool(name="const", bufs=1) as cpool,
            tc.tile_pool(name="stage", bufs=2) as spool,
            tc.tile_pool(name="psum", bufs=PSUM_BUFS, space="PSUM") as ppool,
            tc.tile_pool(name="out", bufs=1) as opool,
        ):
            rhs_sb = cpool.tile([128, 2, N], FP8, tag="rhs", name="rhs_sb")
            lhs_sb = cpool.tile([128, 2, R], FP8, tag="lhs", name="lhs_sb")
            lcorr_sb = cpool.tile([CAP, n_ib * 128], FP8, tag="lc", name="lcorr_sb")
            rcorr_sb = cpool.tile([CAP, n_ib * WINW], FP8, tag="rc", name="rcorr_sb")
            nc.sync.dma_start(rhs_sb[:], rhs_d[:])
            nc.sync.dma_start(lhs_sb[:], lhs_d[:])
            nc.sync.dma_start(lcorr_sb[:], lcorr_d[:])
            nc.sync.dma_start(rcorr_sb[:], rcorr_d[:])

            res = opool.tile([128, n_ops, 2], F16, tag="res", name="res")
            acc = opool.tile([128, n_ops], F32, tag="acc", name="acc")

            def scan(op_id, in0_ap, n_elem):
                # custom dual min/max at REGULAR 1x (perf modes are
                # firmware-broken); pair word -> row min, accum -> row max
                r = nc.vector._custom_dve(
                    MINMAX_OP,
                    out=res[:, op_id : op_id + 1, :].broadcast_to(
                        (128, n_elem // 2, 2)
                    ),
                    in0=in0_ap,
                    s1=MIN_SEED,
                    accum_out=acc[:, op_id : op_id + 1],
                )
                r.ins.perf_max = 0
                return r

            def emit_pe(ib, jc, pt):
                pieces = _corr_pieces_local(ib, N)
                ibsl = slice(ib * 128, (ib + 1) * 128)
                cj0 = jc * JCH
                my_pieces = [
                    (so, a, b) for (so, a, b) in pieces if cj0 <= a < cj0 + JCH
                ]
                corr_banks = {a // SEG for (_, a, b) in my_pieces}
                if "pe" in bench_skip:
                    nc.scalar.memzero(pt[:])
                    return
                for sg in range(n_seg):
                    j0 = cj0 + sg * SEG
                    nc.tensor.matmul(
                        pt[:, sg * SEG : (sg + 1) * SEG],
                        lhs_sb[:, :, ibsl],
                        rhs_sb[:, :, j0 : j0 + SEG],
                        start=True,
                        stop=(j0 // SEG) not in corr_banks,
                        perf_mode=mybir.MatmulPerfMode.DoubleRow,
                        skip_group_check=True,
                    )
                for so, a, b in my_pieces:
                    nc.tensor.matmul(
                        pt[:, a - cj0 : b - cj0],
                        lcorr_sb[:, ibsl],
                        rcorr_sb[:, ib * WINW + so : ib * WINW + so + (b - a)],
                        start=False,
                        stop=True,
                        skip_group_check=True,
                    )

            def gp_tree(st, t, scratch, alu):
                """Pairwise-combine tree on GpSimd from width t down to
                <= GP_TAIL, bouncing between scratch halves.  Returns the
                (offset, width) of the tail in scratch."""
                half = scratch.shape[-1] // 2
                off, w = 0, t // 2
                nc.gpsimd.tensor_tensor(
                    scratch[:, off : off + w], st[:, :w], st[:, w : 2 * w], alu
                )
                while w > GP_TAIL:
                    noff = half if off == 0 else 0
                    nw = w // 2
                    nc.gpsimd.tensor_tensor(
                        scratch[:, noff : noff + nw],
                        scratch[:, off : off + nw],
                        scratch[:, off + nw : off + w],
                        alu,
                    )
                    off, w = noff, nw
                return off, w

            def trace_body():
                tails = []  # deferred DVE tail scans: (op_id, scratch, off, w)
                gp_idx = 0
                for ib in trace_order:
                    mode, s = modes[ib % len(modes)]
                    base = op_base[ib]
                    st = None
                    if mode == "row":
                        st = spool.tile(
                            [128, N], F16, tag="strow", name=f"st{ib}"
                        )
                    elif mode == "gp":
                        st = spool.tile(
                            [128, s * JCH],
                            F16,
                            tag=f"stgp{gp_idx}",
                            bufs=1,
                            name=f"stgp{ib}",
                        )
                    for jc in range(n_jc):
                        pt = ppool.tile(
                            [128, JCH], F32, tag="pt", name=f"pt{ib}_{jc}"
                        )
                        emit_pe(ib, jc, pt)
                        if mode == "direct":
                            if "dve" not in bench_skip:
                                scan(base + jc, pt[:], JCH)
                        elif mode == "row":
                            if "act" not in bench_skip:
                                nc.scalar.copy(
                                    st[:, jc * JCH : (jc + 1) * JCH], pt[:]
                                )
                            if jc == n_jc - 1 and "dve" not in bench_skip:
                                scan(base, st[:], N)
                        else:  # gp
                            if jc < s:
                                if "act" not in bench_skip:
                                    nc.scalar.copy(
                                        st[:, jc * JCH : (jc + 1) * JCH], pt[:]
                                    )
                            else:
                                if "dve" not in bench_skip:
                                    scan(base + (jc - s), pt[:], JCH)
                    if mode == "gp" and "dve" not in bench_skip:
                        t = s * JCH
                        gmin = spool.tile(
                            [128, t], F16, tag=f"gmin{gp_idx}", bufs=1,
                            name=f"gmin{ib}",
                        )
                        gmax = spool.tile(
                            [128, t], F16, tag=f"gmax{gp_idx}", bufs=1,
                            name=f"gmax{ib}",
                        )
                        omin = gp_tree(st, t, gmin, mybir.AluOpType.min)
                        omax = gp_tree(st, t, gmax, mybir.AluOpType.max)
                        nd = n_jc - s
                        tails.append((base + nd, gmin, omin))
                        tails.append((base + nd + 1, gmax, omax))
                        gp_idx += 1
                for op_id, scr, (off, w) in tails:
                    scan(op_id, scr[:, off : off + w], w)

            if repeat == 1:
                trace_body()
            else:
                with tc.For_i(0, repeat, 1):
                    trace_body()
            if "dve" not in bench_skip:
                nc.sync.dma_start(res_d[:], res[:])
                nc.sync.dma_start(acc_d[:], acc[:])
    nc.compile()
    return nc


# --------------------------------------------------------------------------
# Host side
# --------------------------------------------------------------------------


def _avg_nonzero(losses):
    nz = np.count_nonzero(losses > 0)
    return float(np.sum(losses) / nz) if nz > 0 else 0.0


def _pack_fp8(a2d):
    """[256, M] -> DoubleRow-packed [128, 2, M] fp8e4m3."""
    d, m = a2d.shape
    assert d == 256
    return np.ascontiguousarray(
        a2d.reshape(2, 128, m).transpose(1, 0, 2)
    ).astype(ml_dtypes.float8_e4m3)


def _extract_labels(pos, neg):
    """Recover labels from the masks; None if they lack label structure."""
    packed = np.packbits(neg, axis=1)
    key = packed.view([("", f"V{packed.shape[1]}")]).ravel()
    _, labels = np.unique(key, return_inverse=True)
    same = labels[:, None] == labels[None, :]
    if np.array_equal(neg, ~same):
        np.fill_diagonal(same, False)
        if np.array_equal(pos, same):
            return labels
    return None


def _host_reference(x, pos, neg):
    """Exact numpy fallback for non-label-structured masks."""
    x = np.asarray(x, np.float32)
    sq = np.sum(x * x, axis=1)
    d2 = sq[:, None] + sq[None, :] - 2.0 * (x @ x.T)
    dist = np.sqrt(np.maximum(d2, 1e-12), dtype=np.float32)
    has_pos = pos.any(axis=1)
    has_neg = neg.any(axis=1)
    valid = has_pos & has_neg
    hp = np.max(np.where(pos, dist, -1.0), axis=1)
    hn = np.min(np.where(neg, dist, 1e10), axis=1)
    pl = np.where(valid, np.maximum(hp - POS_MARGIN, 0.0), 0.0)
    nl = np.where(valid, np.maximum(NEG_MARGIN - hn, 0.0), 0.0)
    return np.float32(_avg_nonzero(pl) + _avg_nonzero(nl))


def _prep_inputs(embeddings, positives_mask, negatives_mask, n_cores):
    x = np.asarray(embeddings, dtype=np.float32)
    pos = np.asarray(positives_mask).astype(bool)
    neg = np.asarray(negatives_mask).astype(bool)
    n, d = x.shape
    r = n // n_cores
    if d != 256 or n % (n_cores * 128) != 0:
        return None, {"fallback": True, "x": x, "pos": pos, "neg": neg}

    labels = _extract_labels(pos, neg)
    if labels is None:
        return None, {"fallback": True, "x": x, "pos": pos, "neg": neg}

    perm = np.argsort(labels, kind="stable")
    labels_s = labels[perm]
    starts = np.flatnonzero(np.r_[True, labels_s[1:] != labels_s[:-1]])
    sizes = np.diff(np.r_[starts, n])
    if sizes.max() > CAP:
        return None, {"fallback": True, "x": x, "pos": pos, "neg": neg}
    cls_of_row = np.repeat(np.arange(len(starts)), sizes)

    x64 = x.astype(np.float64)[perm]
    sq = np.sum(x64**2, axis=1)
    sq_max, sq_min = float(sq.max()), float(sq.min())
    s = 0.125
    while 5.0 * s * sq_max - s * sq_min > B_FILL - 14.0 and s > 2.0**-40:
        s *= 0.5

    _, V = np.linalg.eigh(x64.T @ x64)
    xr = x64 @ V[:, 2:]  # [N, 254]; rotation preserves distances
    f8 = ml_dtypes.float8_e4m3
    c = np.sqrt(2.0 * s)
    sqs = (s * sq - A_OFF).astype(np.float32)
    sq_hi = sqs.astype(f8)
    sq_lo = (sqs - sq_hi.astype(np.float32)).astype(f8)

    rhs_aug = np.empty((d, n), dtype=np.float32)
    rhs_aug[: d - 2] = (c * xr.T).astype(f8).astype(np.float32)
    rhs_aug[d - 2] = sq_hi.astype(np.float32)
    rhs_aug[d - 1] = sq_lo.astype(np.float32)
    rhs_full = _pack_fp8(rhs_aug)  # [128, 2, N] in global sorted col order
    lhs_aug_full = np.empty((d, n), dtype=np.float32)
    lhs_aug_full[: d - 2] = (-c * xr.T).astype(f8).astype(np.float32)
    lhs_aug_full[d - 2 :] = 1.0

    n_ib = r // 128
    in_maps = []
    for ci in range(n_cores):
        rows = slice(ci * r, (ci + 1) * r)
        lhs = _pack_fp8(np.ascontiguousarray(lhs_aug_full[:, rows]))
        rhs_ci = np.ascontiguousarray(np.roll(rhs_full, -ci * r, axis=2))
        lcorr = np.zeros((CAP, n_ib * 128), dtype=f8)
        rcorr = np.zeros((CAP, n_ib * WINW), dtype=f8)
        ok = True
        for ib in range(n_ib):
            g0 = (ci * n_ib + ib) * 128
            local = np.unique(cls_of_row[g0 : g0 + 128])
            if len(local) > CAP:
                ok = False
                break
            lmap = np.full(cls_of_row.max() + 1, -1, dtype=np.int64)
            lmap[local] = np.arange(len(local))
            lcorr[lmap[cls_of_row[g0 : g0 + 128]], ib * 128 + np.arange(128)] = 16.0
            for so, a, b in _corr_pieces_local(ib, n):
                gcols = (np.arange(a, b) + ci * r) % n
                cls = cls_of_row[gcols]
                sel = np.flatnonzero(lmap[cls] >= 0)
                rcorr[lmap[cls[sel]], ib * WINW + so + sel] = 8.0
        if not ok:
            return None, {"fallback": True, "x": x, "pos": pos, "neg": neg}
        in_maps.append(
            {"lhs8": lhs, "rhs8": rhs_ci, "lcorr": lcorr, "rcorr": rcorr}
        )
    aux = {
        "fallback": False,
        "sq": sq,
        "s": s,
        "perm": perm,
        "n": n,
        "r": r,
        "has_pos": pos.any(axis=1),
        "has_neg": neg.any(axis=1),
    }
    return in_maps, aux


def _decode(results, aux, n_cores, modes=BLOCK_MODES):
    sq, s, perm = aux["sq"], aux["s"], aux["perm"]
    n, r = aux["n"], aux["r"]
    n_ib = r // 128
    n_jc = n // JCH
    ops = _op_schedule(n_ib, n_jc, modes)

    rmin = np.full(n, np.inf)
    rmax = np.full(n, -np.inf)
    for ci in range(n_cores):
        res = np.asarray(results[ci]["res"], dtype=np.float64)  # [128, n_ops, 2]
        acc = np.asarray(results[ci]["acc"], dtype=np.float64)  # [128, n_ops]
        pair_min = np.minimum(res[:, :, 0], res[:, :, 1])
        for k, (ib, kind) in enumerate(ops):
            rows = slice(ci * r + ib * 128, ci * r + (ib + 1) * 128)
            if kind in ("both", "min"):
                rmin[rows] = np.minimum(rmin[rows], pair_min[:, k])
            if kind in ("both", "max"):
                rmax[rows] = np.maximum(rmax[rows], acc[:, k])

    pos_d2 = (rmax - B_FILL + A_OFF) / s + sq
    neg_d2 = (rmin + A_OFF) / s + sq
    hp = np.sqrt(np.maximum(pos_d2, 1e-12))
    hn = np.sqrt(np.maximum(neg_d2, 1e-12))
    valid = (aux["has_pos"] & aux["has_neg"])[perm]
    pos_loss = np.where(valid, np.maximum(hp - POS_MARGIN, 0.0), 0.0)
    neg_loss = np.where(valid, np.maximum(NEG_MARGIN - hn, 0.0), 0.0)
    return np.float32(_avg_nonzero(pos_loss) + _avg_nonzero(neg_loss))


_NC_CACHE = {}


def _kernel_impl(embeddings, positives_mask, negatives_mask, trace=False):
    x = np.asarray(embeddings)
    n, d = x.shape
    in_maps, aux = _prep_inputs(
        embeddings, positives_mask, negatives_mask, N_CORES
    )
    if aux.get("fallback"):
        return _host_reference(aux["x"], aux["pos"], aux["neg"]), None
    key = (n // N_CORES, n, d)
    if key not in _NC_CACHE:
        _NC_CACHE[key] = build_nc(*key)
    nc = _NC_CACHE[key]
    out = run_bass_kernel_spmd(nc, in_maps, list(range(N_CORES)), trace=trace)
    result = _decode(out.results, aux, N_CORES)
    return result, out


def kernel(embeddings, positives_mask, negatives_mask):
    result, _ = _kernel_impl(embeddings, positives_mask, negatives_mask)
    return result


# revision 19
# speedup vs baseline: 1.1206x; 1.1191x over previous
"""BatchHardContrastiveLoss Trainium2 kernel (8-core SPMD), v4.

Math: with rows/columns class-sorted (labels recovered on the host from the
mask structure), the PE emits ONE biased matrix per anchor tile:

    v[i,j] = s*(d2(i,j) - sq_i) - A + B*same[i,j]

The B*same band (classes are contiguous after sorting, so same(i,j) for a
128-row block lives in a fixed 256-wide diagonal window) is added by a tiny
rank-<=64 one-hot fp8 matmul accumulated onto the main DoubleRow matmul's
PSUM segments.  B > 5*s*sq_max guarantees strict separation, so

    row max of v  = B - A + s*(hardest_pos_d2 - sq_i)   (pos mining, diag incl.)
    row min of v  =   - A + s*(hardest_neg_d2 - sq_i)   (neg mining)

i.e. both reductions are PLAIN unmasked max/min — no mask stream, no mask
DMA (v3 DMA'd a 64 MB u8 mask plane and fed it to the DVE as a second
operand).  Each core's rhs columns are rotated by -core*R so the diagonal
band sits at the same local columns on every core (one SPMD program); block
0 gets an extra wrapped window piece [N-64, N).  Min/max are column-order
invariant so the decode is unaffected.

A custom single-src dual min/max DVE op (ANT_DUAL_MINMAX_SS) computes both
reductions in one 1x pass per PSUM chunk: a stride-0 [min,max] pair word
(min of the pair = row min) plus the fp32 max accumulator (accum_out = row
max).  The op also carries hand-written uop programs for the 2x/4x DVE perf
modes, but the custom-DVE perf-mode dispatch is firmware-broken on this
snapshot (the T1 parity test is xfail; any perf_max>0 faults the engine,
verified down to a trivial delay-only op), so they are never advertised
(PERF_MAX=0).  With modes dead, every engine path bottoms out at the same
1 cell/lane/cycle DVE wall (PSUM port = 32b/cyc; stock TT/reduce trees
re-read intermediates; ACT accumulates sums only; GpSimd has no PSUM port
and its tensor_tensor fails to compile), so the kernel scans all 8.4M
cells/core on the DVE at 1x: 65536 cyc @ 0.96 GHz = 68.3us floor, ~72-74us
measured (v3 baseline: 81.8us; the win is the removed mask stream and
leaner per-op overhead).

Engine roles per core (R=1024 rows x N=8192 cols):
  PE : DoubleRow fp8 matmuls + rank-<=64 band correction (~15% busy)
  DVE: dual min/max scan of each [128, 2048] PSUM chunk (bottleneck)
  ACT/GpSimd: idle ("row"/"gp" BLOCK_MODES exist but don't pay off at 1x)

Host: recovers labels (np.unique of mask rows), sorts, PCA-rotates (exact
for distances), packs fp8, decodes d2, applies margins/AvgNonZero, and
computes validity (has_pos/has_neg) exactly from the input masks.  If the
masks do not have label structure (never for this problem's generator),
falls back to an exact numpy implementation.
"""

import numpy as np
import ml_dtypes

import concourse.bass as bass  # noqa: F401
import concourse.mybir as mybir
import concourse.tile as tile
from concourse import bacc
from concourse import dve_ops as _dvo
from concourse.bass_utils import run_bass_kernel_spmd
from concourse.dve_spec import C1, MaxNeg, Spec, Src0, maxx, minn
from concourse.dve_table_gen import dve_ver_for
from concourse.dve_uop import (
    DISABLE,
    ENABLE,
    AluInp,
    AluOp,
    DelayInp,
    DveOpSpec,
    InpSel,
    OutPath,
    OutSel,
    Trigger,
    UopConfig,
)

N_CORES = 8
JCH = 2048
SEG = 512
PSUM_BUFS = 2
CAP = 64          # max classes per 128-row block AND max class size (asserted)
WINW = 256        # per-block correction window slot width
B_FILL = 128.0    # same-class bias (16*8, fp8-exact)
A_OFF = 16.0      # constant shift folded into the sq rows
MIN_SEED = 3.0e38
POS_MARGIN = 0.2
NEG_MARGIN = 0.2
# NOTE: custom-DVE perf modes (PERF_MAX>0) are firmware-broken on this
# snapshot (T1 incomplete; the engine faults on any mode >= 2x_1p), so all
# DVE scans run REGULAR 1x.  Per-block modes:
#   ("direct", 0): DVE dual-scans the 4 PSUM chunks directly (no copies)
#   ("row", 4):    ACT copies all 4 chunks to fp16 SBUF; DVE scans the full
#                  row in ONE op (amortizes the per-op drain)
#   ("gp", s):     ACT copies s chunks to fp16 SBUF; GpSimd runs pairwise
#                  min/max trees; DVE scans the <=512-wide tails; the other
#                  4-s chunks go direct
# Measured on HW: all-direct ~72-74us, all-row ~75us (statistically tied,
# both near the 68.3us 1x-scan floor); "gp" does not compile (walrus rejects
# nc.gpsimd.tensor_tensor in this toolchain).  Ship all-direct: fewest
# engines, no staging dependencies.
BLOCK_MODES = (("direct", 0),) * 8
PERF_MAX = 0  # custom-DVE perf modes are firmware-broken; keep 0

F32 = mybir.dt.float32
F16 = mybir.dt.float16
FP8 = mybir.dt.float8e4


# --------------------------------------------------------------------------
# Custom DVE op: single-src dual min/max, all four perf modes.
#
# Steady-state semantics (per instruction):
#   out pair word <- [running_min, running_max] every cycle (stride-0 dst);
#     in 1x mode only the lo half is written (running_min), so
#     min(out[0], out[1]) == row min in EVERY mode.
#   accum_out <- row max (fp32, via block-7 a-flop accumulator).
# A 1-cycle seed uop initializes the min recurrence to C1 (s1=+3e38) and the
# max recurrence to MAX_NEG.
# --------------------------------------------------------------------------


def _seed_from(steady: UopConfig, min_blk: int) -> UopConfig:
    """Seed uop: block min_blk's flop <- C1 (via chain4), block 7's flop <-
    MAX_NEG (via chain5)."""
    u = UopConfig()
    u.inp = list(steady.inp)
    u.inp_enable = list(steady.inp_enable)
    dp = u.datapath_config
    for b in range(min_blk):
        dp[b].pass_through_delay(4, 5)
    dp[min_blk].enable_alu(
        AluOp.BYPASS, AluInp.PREV_DELAY_4, AluInp.PREV_DELAY_4
    ).pass_through_delay(5)
    for b in range(min_blk + 1, 7):
        dp[b].pass_through_delay(5)
    dp[7].enable_alu(AluOp.BYPASS, AluInp.PREV_DELAY_5, AluInp.PREV_DELAY_5)
    dp[7].alu_out_a_enable = ENABLE
    for p in OutPath:
        u.out_enable[p] = DISABLE
    u.require_inp0 = DISABLE
    u.require_inp1 = DISABLE
    u.repeat_count = 1
    u.trigger = (Trigger.COUNT, Trigger.NONE, Trigger.NONE)
    u.next_uop = (1, 0, 0)
    u.accum_enabled = ENABLE
    return u


def _steady_common(u: UopConfig):
    u.enable_input(InpSel.CONST_1, 5)   # chain4 seed value (block-0 view only)
    u.enable_input(InpSel.MAX_NEG, 6)   # chain5 seed value
    u.require_inp0 = ENABLE
    u.require_inp1 = DISABLE
    u.trigger = (Trigger.SRC_TENSOR_DONE, Trigger.NONE, Trigger.NONE)
    u.next_uop = (0, 0, 0)
    u.accum_enabled = ENABLE


def _mk_1x() -> list[UopConfig]:
    u = UopConfig()
    u.enable_input(InpSel.SRC_0, 1)  # chain0 = x
    _steady_common(u)
    dp = u.datapath_config
    # b0: min_rec = MIN(self, x); chain4 <- new min; carry x on chain0
    dp[0].enable_alu(
        AluOp.MIN, AluInp.CURR_ALU_OUT, AluInp.PREV_DELAY_0
    ).pass_through_delay(0).enable_delay_from_src(DelayInp.CURR_ALU_OUT, 4)
    for b in range(1, 7):
        dp[b].pass_through_delay(0, 4)
    # b7: max_rec = MAX(self, x); a-flop -> accum
    dp[7].enable_alu(
        AluOp.MAX, AluInp.CURR_ALU_OUT, AluInp.PREV_DELAY_0
    ).pass_through_delay(4)
    dp[7].alu_out_a_enable = ENABLE
    u.enable_output(OutSel.DELAY_4, OutPath.WR0_LO)
    return [_seed_from(u, 0), u]


def _mk_2x(second: InpSel) -> list[UopConfig]:
    u = UopConfig()
    u.enable_input(InpSel.SRC_0, 1)  # chain0 = a
    u.enable_input(second, 2)        # chain1 = b
    _steady_common(u)
    dp = u.datapath_config
    # b0: m = MIN(a, b); carry a,b
    dp[0].enable_alu(
        AluOp.MIN, AluInp.PREV_DELAY_0, AluInp.PREV_DELAY_1
    ).pass_through_delay(0, 1)
    # b1: min_rec = MIN(self, m); chain4 <- new min
    dp[1].enable_alu(
        AluOp.MIN, AluInp.CURR_ALU_OUT, AluInp.PREV_ALU_OUT
    ).pass_through_delay(0, 1).enable_delay_from_src(DelayInp.CURR_ALU_OUT, 4)
    for b in range(2, 6):
        dp[b].pass_through_delay(0, 1, 4)
    # b6: M = MAX(a, b)
    dp[6].enable_alu(
        AluOp.MAX, AluInp.PREV_DELAY_0, AluInp.PREV_DELAY_1
    ).pass_through_delay(4)
    # b7: max_rec = MAX(self, M); a-flop -> accum
    dp[7].enable_alu(
        AluOp.MAX, AluInp.CURR_ALU_OUT, AluInp.PREV_ALU_OUT
    ).pass_through_delay(4)
    dp[7].alu_out_a_enable = ENABLE
    u.enable_output(OutSel.DELAY_4, OutPath.WR0_LO)
    u.enable_output(OutSel.ALU_OUT, OutPath.WR0_HI)
    return [_seed_from(u, 1), u]


def _mk_4x() -> list[UopConfig]:
    u = UopConfig()
    u.enable_input(InpSel.SRC_0, 1)     # chain0 = a
    u.enable_input(InpSel.SRC_0_HI, 2)  # chain1 = b
    u.enable_input(InpSel.SRC_1, 3)     # chain2 = c
    u.enable_input(InpSel.SRC_1_HI, 4)  # chain3 = d
    _steady_common(u)
    dp = u.datapath_config
    # min tree: b0 m01, b1 m23 (capture m01->ch4), b2 mq, b3 min_rec
    dp[0].enable_alu(
        AluOp.MIN, AluInp.PREV_DELAY_0, AluInp.PREV_DELAY_1
    ).pass_through_delay(0, 1, 2, 3)
    dp[1].enable_alu(
        AluOp.MIN, AluInp.PREV_DELAY_2, AluInp.PREV_DELAY_3
    ).pass_through_delay(0, 1, 2, 3).enable_delay_from_src(
        DelayInp.PREV_ALU_OUT, 4
    )
    dp[2].enable_alu(
        AluOp.MIN, AluInp.PREV_ALU_OUT, AluInp.PREV_DELAY_4
    ).pass_through_delay(0, 1, 2, 3)
    dp[3].enable_alu(
        AluOp.MIN, AluInp.CURR_ALU_OUT, AluInp.PREV_ALU_OUT
    ).pass_through_delay(0, 1, 2, 3).enable_delay_from_src(
        DelayInp.CURR_ALU_OUT, 4
    )
    # max tree: b4 M01, b5 M23 (capture M01->ch5), b6 Mq, b7 max_rec
    dp[4].enable_alu(
        AluOp.MAX, AluInp.PREV_DELAY_0, AluInp.PREV_DELAY_1
    ).pass_through_delay(2, 3, 4)
    dp[5].enable_alu(
        AluOp.MAX, AluInp.PREV_DELAY_2, AluInp.PREV_DELAY_3
    ).pass_through_delay(4).enable_delay_from_src(DelayInp.PREV_ALU_OUT, 5)
    dp[6].enable_alu(
        AluOp.MAX, AluInp.PREV_ALU_OUT, AluInp.PREV_DELAY_5
    ).pass_through_delay(4)
    dp[7].enable_alu(
        AluOp.MAX, AluInp.CURR_ALU_OUT, AluInp.PREV_ALU_OUT
    ).pass_through_delay(4)
    dp[7].alu_out_a_enable = ENABLE
    u.enable_output(OutSel.DELAY_4, OutPath.WR0_LO)
    u.enable_output(OutSel.ALU_OUT, OutPath.WR0_HI)
    u.enable_output(OutSel.DELAY_4, OutPath.WR1_LO)
    u.enable_output(OutSel.ALU_OUT, OutPath.WR1_HI)
    return [_seed_from(u, 3), u]


def _minmax_reference(in0, in1, c0, c1, c2):
    """CoreSim reference (1x semantics).  out = running min (the stride-0
    dst keeps only the final pair; min(lo,hi)=row min in every mode);
    accum = row max."""
    x = np.asarray(in0, np.float32)
    x2 = x.reshape(x.shape[0], -1)
    c1r = np.asarray(c1, np.float32).reshape(-1, 1)
    run = np.minimum.accumulate(np.minimum(x2, c1r), axis=-1)
    acc = np.max(x2, axis=-1, keepdims=True)
    return run, acc


def _register_minmax_op(name: str = "ANT_DUAL_MINMAX_SS"):
    for op in _dvo.OPS:
        if op.name == name:
            return op
    row = _dvo._CUSTOM_DVE_ROW_BASE + len(_dvo.OPS)
    assert row < 0x20, "custom-DVE sub-opcode rows exhausted"
    _dvo._SUB_OPCODE_FOR_NAME[name] = row
    ver = dve_ver_for("TRN2")
    u1 = _mk_1x()
    u2 = _mk_2x(InpSel.SRC_0_HI)
    u2p = _mk_2x(InpSel.SRC_1)
    u4 = _mk_4x()
    for us in (u1, u2, u2p, u4):
        for u in us:
            u.validate(ver)
    opspec = DveOpSpec(
        name=name,
        opcode=row,
        uops=u1,
        uops_2x=u2,
        uops_2x_2p=u2p,
        uops_4x=u4,
        perf_max=3,
        rd1_en=False,
    )
    sha = opspec.sha(ver)
    carrier = Spec(
        body=minn(Src0, C1),
        accum=maxx,
        accum_init=MaxNeg,
        reference=_minmax_reference,
    )
    op = _dvo.DveOp(
        name, carrier, subdim=False, uops_sha={ver: sha}, perf_en={ver: True}
    )
    _dvo._COMPILE_CACHE[(name, ver)] = opspec
    _dvo.OPS.append(op)
    _dvo.CUSTOM_DVE_SPECS[name] = carrier
    return op


MINMAX_OP = _register_minmax_op()


# --------------------------------------------------------------------------
# Schedules shared by trace, host prep, and decode (all in the per-core
# LOCAL column space: core ci sees global sorted columns rolled by -ci*R).
# --------------------------------------------------------------------------


def _corr_pieces_local(ib: int, n: int):
    """Correction pieces for local row-block ib as (slot_off, a, b): local
    columns [a, b) mapped from rcorr slot columns [slot_off, slot_off+b-a).
    Pieces never cross a 512-col bank boundary.  Block 0 wraps: its window
    is [0, 192) plus the wrapped tail [n-64, n)."""
    if ib == 0:
        return [(0, 0, 192), (192, n - 64, n)]
    w0 = ib * 128 - 64
    w1 = w0 + WINW
    pieces = []
    a = w0
    while a < w1:
        b = min(w1, (a // SEG + 1) * SEG)
        pieces.append((a - w0, a, b))
        a = b
    return pieces


GP_TAIL = 512  # GpSimd trees halve down to this width; DVE scans the tail


def _op_schedule(n_ib: int, n_jc: int, modes):
    """Per-block result columns: [(ib, kind)] with kind in
    "both" (pair->min AND acc->max), "min" (pair only), "max" (acc only)."""
    ops = []
    for ib in range(n_ib):
        mode, s = modes[ib % len(modes)]
        if mode == "direct":
            ops += [(ib, "both")] * n_jc
        elif mode == "row":
            ops.append((ib, "both"))
        else:  # gp
            ops += [(ib, "both")] * (n_jc - s)  # direct chunks
            ops.append((ib, "min"))
            ops.append((ib, "max"))
    return ops


# --------------------------------------------------------------------------
# Device program (identical for every core)
# --------------------------------------------------------------------------


def build_nc(R, N, D, modes=BLOCK_MODES, repeat=1, bench_skip=()):
    assert R % 128 == 0 and N % JCH == 0 and D == 256
    n_ib = R // 128
    n_jc = N // JCH
    n_seg = JCH // SEG
    ops = _op_schedule(n_ib, n_jc, modes)
    n_ops = len(ops)
    # op-id bases per block, mirroring _op_schedule's emission order
    op_base = {}
    k = 0
    for ib in range(n_ib):
        op_base[ib] = k
        mode, s = modes[ib % len(modes)]
        k += n_jc if mode == "direct" else (1 if mode == "row" else (n_jc - s) + 2)
    gp_blocks = [ib for ib in range(n_ib) if modes[ib % len(modes)][0] == "gp"]
    # trace order: gp first (GpSimd starts early), then direct (DVE starts
    # early), then row blocks
    trace_order = (
        gp_blocks
        + [ib for ib in range(n_ib) if modes[ib % len(modes)][0] == "direct"]
        + [ib for ib in range(n_ib) if modes[ib % len(modes)][0] == "row"]
    )

    nc = bacc.Bacc(None, target_bir_lowering=False)
    lhs_d = nc.dram_tensor("lhs8", [128, 2, R], FP8, kind="ExternalInput")
    rhs_d = nc.dram_tensor("rhs8", [128, 2, N], FP8, kind="ExternalInput")
    lcorr_d = nc.dram_tensor("lcorr", [CAP, n_ib * 128], FP8, kind="ExternalInput")
    rcorr_d = nc.dram_tensor("rcorr", [CAP, n_ib * WINW], FP8, kind="ExternalInput")
    res_d = nc.dram_tensor("res", [128, n_ops, 2], F16, kind="ExternalOutput")
    acc_d = nc.dram_tensor("acc", [128, n_ops], F32, kind="ExternalOutput")

    with tile.TileContext(nc) as tc:
        with (
            tc.tile_pool(name="const", bufs=1) as cpool,
            tc.tile_pool(name="stage", bufs=2) as spool,
            tc.tile_pool(name="psum", bufs=PSUM_BUFS, space="PSUM") as ppool,
            tc.tile_pool(name="out", bufs=1) as opool,
        ):
            rhs_sb = cpool.tile([128, 2, N], FP8, tag="rhs", name="rhs_sb")
            lhs_sb = cpool.tile([128, 2, R], FP8, tag="lhs", name="lhs_sb")
            lcorr_sb = cpool.tile([CAP, n_ib * 128], FP8, tag="lc", name="lcorr_sb")
            rcorr_sb = cpool.tile([CAP, n_ib * WINW], FP8, tag="rc", name="rcorr_sb")
            nc.sync.dma_start(rhs_sb[:], rhs_d[:])
            nc.sync.dma_start(lhs_sb[:], lhs_d[:])
            nc.sync.dma_start(lcorr_sb[:], lcorr_d[:])
            nc.sync.dma_start(rcorr_sb[:], rcorr_d[:])

            res = opool.tile([128, n_ops, 2], F16, tag="res", name="res")
            acc = opool.tile([128, n_ops], F32, tag="acc", name="acc")

            def scan(op_id, in0_ap, n_elem):
                # custom dual min/max at REGULAR 1x (perf modes are
                # firmware-broken); pair word -> row min, accum -> row max
                r = nc.vector._custom_dve(
                    MINMAX_OP,
                    out=res[:, op_id : op_id + 1, :].broadcast_to(
                        (128, n_elem // 2, 2)
                    ),
                    in0=in0_ap,
                    s1=MIN_SEED,
                    accum_out=acc[:, op_id : op_id + 1],
                )
                r.ins.perf_max = 0
                return r

            def emit_pe(ib, jc, pt):
                pieces = _corr_pieces_local(ib, N)
                ibsl = slice(ib * 128, (ib + 1) * 128)
                cj0 = jc * JCH
                my_pieces = [
                    (so, a, b) for (so, a, b) in pieces if cj0 <= a < cj0 + JCH
                ]
                corr_banks = {a // SEG for (_, a, b) in my_pieces}
                if "pe" in bench_skip:
                    nc.scalar.memzero(pt[:])
                    return
                for sg in range(n_seg):
                    j0 = cj0 + sg * SEG
                    nc.tensor.matmul(
                        pt[:, sg * SEG : (sg + 1) * SEG],
                        lhs_sb[:, :, ibsl],
                        rhs_sb[:, :, j0 : j0 + SEG],
                        start=True,
                        stop=(j0 // SEG) not in corr_banks,
                        perf_mode=mybir.MatmulPerfMode.DoubleRow,
                        skip_group_check=True,
                    )
                for so, a, b in my_pieces:
                    nc.tensor.matmul(
                        pt[:, a - cj0 : b - cj0],
                        lcorr_sb[:, ibsl],
                        rcorr_sb[:, ib * WINW + so : ib * WINW + so + (b - a)],
                        start=False,
                        stop=True,
                        skip_group_check=True,
                    )

            def gp_tree(st, t, scratch, alu):
                """Pairwise-combine tree on GpSimd from width t down to
                <= GP_TAIL, bouncing between scratch halves.  Returns the
                (offset, width) of the tail in scratch."""
                half = scratch.shape[-1] // 2
                off, w = 0, t // 2
                nc.gpsimd.tensor_tensor(
                    scratch[:, off : off + w], st[:, :w], st[:, w : 2 * w], alu
                )
                while w > GP_TAIL:
                    noff = half if off == 0 else 0
                    nw = w // 2
                    nc.gpsimd.tensor_tensor(
                        scratch[:, noff : noff + nw],
                        scratch[:, off : off + nw],
                        scratch[:, off + nw : off + w],
                        alu,
                    )
                    off, w = noff, nw
                return off, w

            def trace_body():
                tails = []  # deferred DVE tail scans: (op_id, scratch, off, w)
                gp_idx = 0
                for ib in trace_order:
                    mode, s = modes[ib % len(modes)]
                    base = op_base[ib]
                    st = None
                    if mode == "row":
                        st = spool.tile(
                            [128, N], F16, tag="strow", name=f"st{ib}"
                        )
                    elif mode == "gp":
                        st = spool.tile(
                            [128, s * JCH],
                            F16,
                            tag=f"stgp{gp_idx}",
                            bufs=1,
                            name=f"stgp{ib}",
                        )
                    for jc in range(n_jc):
                        pt = ppool.tile(
                            [128, JCH], F32, tag="pt", name=f"pt{ib}_{jc}"
                        )
                        emit_pe(ib, jc, pt)
                        if mode == "direct":
                            if "dve" not in bench_skip:
                                scan(base + jc, pt[:], JCH)
                        elif mode == "row":
                            if "act" not in bench_skip:
                                nc.scalar.copy(
                                    st[:, jc * JCH : (jc + 1) * JCH], pt[:]
                                )
                            if jc == n_jc - 1 and "dve" not in bench_skip:
                                scan(base, st[:], N)
                        else:  # gp
                            if jc < s:
                                if "act" not in bench_skip:
                                    nc.scalar.copy(
                                        st[:, jc * JCH : (jc + 1) * JCH], pt[:]
                                    )
                            else:
                                if "dve" not in bench_skip:
                                    scan(base + (jc - s), pt[:], JCH)
                    if mode == "gp" and "dve" not in bench_skip:
                        t = s * JCH
                        gmin = spool.tile(
                            [128, t], F16, tag=f"gmin{gp_idx}", bufs=1,
                            name=f"gmin{ib}",
                        )
                        gmax = spool.tile(
                            [128, t], F16, tag=f"gmax{gp_idx}", bufs=1,
                            name=f"gmax{ib}",
                        )
                        omin = gp_tree(st, t, gmin, mybir.AluOpType.min)
                        omax = gp_tree(st, t, gmax, mybir.AluOpType.max)
                        nd = n_jc - s
                        tails.append((base + nd, gmin, omin))
                        tails.append((base + nd + 1, gmax, omax))
                        gp_idx += 1
                for op_id, scr, (off, w) in tails:
                    scan(op_id, scr[:, off : off + w], w)

            if repeat == 1:
                trace_body()
            else:
                with tc.For_i(0, repeat, 1):
                    trace_body()
            if "dve" not in bench_skip:
                nc.sync.dma_start(res_d[:], res[:])
                nc.sync.dma_start(acc_d[:], acc[:])
    nc.compile()
    return nc


# --------------------------------------------------------------------------
# Host side
# --------------------------------------------------------------------------


def _avg_nonzero(losses):
    nz = np.count_nonzero(losses > 0)
    return float(np.sum(losses) / nz) if nz > 0 else 0.0


def _pack_fp8(a2d):
    """[256, M] -> DoubleRow-packed [128, 2, M] fp8e4m3."""
    d, m = a2d.shape
    assert d == 256
    return np.ascontiguousarray(
        a2d.reshape(2, 128, m).transpose(1, 0, 2)
    ).astype(ml_dtypes.float8_e4m3)


def _extract_labels(pos, neg):
    """Recover labels from the masks; None if they lack label structure."""
    packed = np.packbits(neg, axis=1)
    key = packed.view([("", f"V{packed.shape[1]}")]).ravel()
    _, labels = np.unique(key, return_inverse=True)
    same = labels[:, None] == labels[None, :]
    if np.array_equal(neg, ~same):
        np.fill_diagonal(same, False)
        if np.array_equal(pos, same):
            return labels
    return None


def _host_reference(x, pos, neg):
    """Exact numpy fallback for non-label-structured masks."""
    x = np.asarray(x, np.float32)
    sq = np.sum(x * x, axis=1)
    d2 = sq[:, None] + sq[None, :] - 2.0 * (x @ x.T)
    dist = np.sqrt(np.maximum(d2, 1e-12), dtype=np.float32)
    has_pos = pos.any(axis=1)
    has_neg = neg.any(axis=1)
    valid = has_pos & has_neg
    hp = np.max(np.where(pos, dist, -1.0), axis=1)
    hn = np.min(np.where(neg, dist, 1e10), axis=1)
    pl = np.where(valid, np.maximum(hp - POS_MARGIN, 0.0), 0.0)
    nl = np.where(valid, np.maximum(NEG_MARGIN - hn, 0.0), 0.0)
    return np.float32(_avg_nonzero(pl) + _avg_nonzero(nl))


def _prep_inputs(embeddings, positives_mask, negatives_mask, n_cores):
    x = np.asarray(embeddings, dtype=np.float32)
    pos = np.asarray(positives_mask).astype(bool)
    neg = np.asarray(negatives_mask).astype(bool)
    n, d = x.shape
    r = n // n_cores
    if d != 256 or n % (n_cores * 128) != 0:
        return None, {"fallback": True, "x": x, "pos": pos, "neg": neg}

    labels = _extract_labels(pos, neg)
    if labels is None:
        return None, {"fallback": True, "x": x, "pos": pos, "neg": neg}

    perm = np.argsort(labels, kind="stable")
    labels_s = labels[perm]
    starts = np.flatnonzero(np.r_[True, labels_s[1:] != labels_s[:-1]])
    sizes = np.diff(np.r_[starts, n])
    if sizes.max() > CAP:
        return None, {"fallback": True, "x": x, "pos": pos, "neg": neg}
    cls_of_row = np.repeat(np.arange(len(starts)), sizes)

    x64 = x.astype(np.float64)[perm]
    sq = np.sum(x64**2, axis=1)
    sq_max, sq_min = float(sq.max()), float(sq.min())
    s = 0.125
    while 5.0 * s * sq_max - s * sq_min > B_FILL - 14.0 and s > 2.0**-40:
        s *= 0.5

    _, V = np.linalg.eigh(x64.T @ x64)
    xr = x64 @ V[:, 2:]  # [N, 254]; rotation preserves distances
    f8 = ml_dtypes.float8_e4m3
    c = np.sqrt(2.0 * s)
    sqs = (s * sq - A_OFF).astype(np.float32)
    sq_hi = sqs.astype(f8)
    sq_lo = (sqs - sq_hi.astype(np.float32)).astype(f8)

    rhs_aug = np.empty((d, n), dtype=np.float32)
    rhs_aug[: d - 2] = (c * xr.T).astype(f8).astype(np.float32)
    rhs_aug[d - 2] = sq_hi.astype(np.float32)
    rhs_aug[d - 1] = sq_lo.astype(np.float32)
    rhs_full = _pack_fp8(rhs_aug)  # [128, 2, N] in global sorted col order
    lhs_aug_full = np.empty((d, n), dtype=np.float32)
    lhs_aug_full[: d - 2] = (-c * xr.T).astype(f8).astype(np.float32)
    lhs_aug_full[d - 2 :] = 1.0

    n_ib = r // 128
    in_maps = []
    for ci in range(n_cores):
        rows = slice(ci * r, (ci + 1) * r)
        lhs = _pack_fp8(np.ascontiguousarray(lhs_aug_full[:, rows]))
        rhs_ci = np.ascontiguousarray(np.roll(rhs_full, -ci * r, axis=2))
        lcorr = np.zeros((CAP, n_ib * 128), dtype=f8)
        rcorr = np.zeros((CAP, n_ib * WINW), dtype=f8)
        ok = True
        for ib in range(n_ib):
            g0 = (ci * n_ib + ib) * 128
            local = np.unique(cls_of_row[g0 : g0 + 128])
            if len(local) > CAP:
                ok = False
                break
            lmap = np.full(cls_of_row.max() + 1, -1, dtype=np.int64)
            lmap[local] = np.arange(len(local))
            lcorr[lmap[cls_of_row[g0 : g0 + 128]], ib * 128 + np.arange(128)] = 16.0
            for so, a, b in _corr_pieces_local(ib, n):
                gcols = (np.arange(a, b) + ci * r) % n
                cls = cls_of_row[gcols]
                sel = np.flatnonzero(lmap[cls] >= 0)
                rcorr[lmap[cls[sel]], ib * WINW + so + sel] = 8.0
        if not ok:
            return None, {"fallback": True, "x": x, "pos": pos, "neg": neg}
        in_maps.append(
            {"lhs8": lhs, "rhs8": rhs_ci, "lcorr": lcorr, "rcorr": rcorr}
        )
    aux = {
        "fallback": False,
        "sq": sq,
        "s": s,
        "perm": perm,
        "n": n,
        "r": r,
        "has_pos": pos.any(axis=1),
        "has_neg": neg.any(axis=1),
    }
    return in_maps, aux


def _decode(results, aux, n_cores, modes=BLOCK_MODES):
    sq, s, perm = aux["sq"], aux["s"], aux["perm"]
    n, r = aux["n"], aux["r"]
    n_ib = r // 128
    n_jc = n // JCH
    ops = _op_schedule(n_ib, n_jc, modes)

    rmin = np.full(n, np.inf)
    rmax = np.full(n, -np.inf)
    for ci in range(n_cores):
        res = np.asarray(results[ci]["res"], dtype=np.float64)  # [128, n_ops, 2]
        acc = np.asarray(results[ci]["acc"], dtype=np.float64)  # [128, n_ops]
        pair_min = np.minimum(res[:, :, 0], res[:, :, 1])
        for k, (ib, kind) in enumerate(ops):
            rows = slice(ci * r + ib * 128, ci * r + (ib + 1) * 128)
            if kind in ("both", "min"):
                rmin[rows] = np.minimum(rmin[rows], pair_min[:, k])
            if kind in ("both", "max"):
                rmax[rows] = np.maximum(rmax[rows], acc[:, k])

    pos_d2 = (rmax - B_FILL + A_OFF) / s + sq
    neg_d2 = (rmin + A_OFF) / s + sq
    hp = np.sqrt(np.maximum(pos_d2, 1e-12))
    hn = np.sqrt(np.maximum(neg_d2, 1e-12))
    valid = (aux["has_pos"] & aux["has_neg"])[perm]
    pos_loss = np.where(valid, np.maximum(hp - POS_MARGIN, 0.0), 0.0)
    neg_loss = np.where(valid, np.maximum(NEG_MARGIN - hn, 0.0), 0.0)
    return np.float32(_avg_nonzero(pos_loss) + _avg_nonzero(neg_loss))


_NC_CACHE = {}


def _kernel_impl(embeddings, positives_mask, negatives_mask, trace=False):
    x = np.asarray(embeddings)
    n, d = x.shape
    in_maps, aux = _prep_inputs(
        embeddings, positives_mask, negatives_mask, N_CORES
    )
    if aux.get("fallback"):
        return _host_reference(aux["x"], aux["pos"], aux["neg"]), None
    key = (n // N_CORES, n, d)
    if key not in _NC_CACHE:
        _NC_CACHE[key] = build_nc(*key)
    nc = _NC_CACHE[key]
    out = run_bass_kernel_spmd(nc, in_maps, list(range(N_CORES)), trace=trace)
    result = _decode(out.results, aux, N_CORES)
    return result, out


def kernel(embeddings, positives_mask, negatives_mask):
    result, _ = _kernel_impl(embeddings, positives_mask, negatives_mask)
    return result
